# revision 1
# baseline (speedup 1.0000x reference)
"""Trainium2 Bass kernel for nn_FC_89094801588783.

Computes, for x[B=16, N=8192, Fin=256], W[256,256], b[256], gamma[256], beta[256]:
    y = x @ W.T + b                       (per-token Linear)
    per-sample BatchNorm over N (biased var), then gamma/beta affine.

Sharding: data-parallel over B across 8 NeuronCores (2 samples per core).
Each core runs one Bass/Tile kernel over its [16384, 256] token slab.

Per-core pipeline (matmuls in float32r = full-rate fp32-rounded):
  - DMA x in [128,4,256] tiles.
  - PE transposes x tiles -> xT [fin,tok] PSUM; ACT evacuates (casts f32r).
  - y^T = W^T-blocks (stationary) @ xT, N=512 moving -> PSUM.
  - DVE bn_stats on the y^T PSUM tile (per-feature mean/M2 over tokens);
    ACT evacuates y^T to SBUF as f32r.
  - Finalize per sample: bn_aggr -> mean/var -> k = gamma*rsqrt(var+eps),
    s2 = beta/k - mean; build diag(k), diag(s2) tiles (f32r) and k_bcast
    via a ones-matmul.
  - Output: PE transposes y^T blocks back to [tok, fout] PSUM, a ones@diag(s2)
    matmul adds the shift, DVE multiplies by k_bcast on evacuation, DMA out.
"""
import sys

sys.path.insert(0, "/opt/trn_rl_repo")

import numpy as np

_NC_CACHE = {}

B, N, F = 16, 8192, 256
CORES = 8
SPB = B // CORES          # samples per core = 2
TOK = SPB * N             # tokens per core = 16384
P = 128
GROUPS = N // 512         # 16 groups of 512 tokens per sample
EPS = 1e-5


def _build_nc():
    import concourse.bacc as bacc
    import concourse.tile as tile
    from concourse import mybir
    from concourse.masks import make_identity

    f32 = mybir.dt.float32
    f32r = mybir.dt.float32r
    AF = mybir.ActivationFunctionType

    nc = bacc.Bacc("TRN2")
    x_d = nc.dram_tensor("x", [TOK, F], f32, kind="ExternalInput")
    w_d = nc.dram_tensor("w", [F, F], f32, kind="ExternalInput")
    b_d = nc.dram_tensor("b", [F], f32, kind="ExternalInput")
    g_d = nc.dram_tensor("gamma", [F], f32, kind="ExternalInput")
    be_d = nc.dram_tensor("beta", [F], f32, kind="ExternalInput")
    out_d = nc.dram_tensor("out", [TOK, F], f32, kind="ExternalOutput")

    with tile.TileContext(nc) as tc:
        with (
            tc.tile_pool(name="consts", bufs=1) as consts,
            tc.tile_pool(name="xin", bufs=4) as xin,
            tc.tile_pool(name="xtp", bufs=4) as xtp,
            tc.tile_pool(name="ypool", bufs=48) as ypool,
            tc.tile_pool(name="misc", bufs=2) as misc,
            tc.tile_pool(name="stats", bufs=4) as stats,
            tc.tile_pool(name="outp", bufs=3) as outp,
            tc.tile_pool(name="ps_xt", bufs=3, space="PSUM") as ps_xt,
            tc.tile_pool(name="ps_y", bufs=3, space="PSUM") as ps_y,
            tc.tile_pool(name="ps_o", bufs=2, space="PSUM") as ps_o,
        ):
            # ---------------- constants ----------------
            ident_f = consts.tile([P, P], f32)
            make_identity(nc, ident_f)
            ident_r = consts.tile([P, P], f32r)
            nc.vector.tensor_copy(ident_r[:], ident_f[:])
            ones_f = consts.tile([P, P], f32)
            nc.vector.memset(ones_f, 1.0)
            ones_r = consts.tile([P, P], f32r)
            nc.vector.tensor_copy(ones_r[:], ones_f[:])
            eps_t = consts.tile([P, 1], f32)
            nc.vector.memset(eps_t, EPS)

            # -------- prefetch first x tiles (head latency) --------
            xpre = {}
            for gi in (0, 2):
                xt0 = xin.tile([P, 8, F], f32r, tag="xnat", name=f"xpre{gi}")
                nc.gpsimd.dma_start(
                    out=xt0[:],
                    in_=x_d[gi * 512:(gi * 512) + 1024, :].rearrange(
                        "(t p) f -> p t f", p=P),
                )
                xpre[gi] = xt0

            w_sb = consts.tile([P, 2, F], f32)
            nc.sync.dma_start(out=w_sb[:], in_=w_d.rearrange("(a p) f -> p a f", p=P))
            b_col = consts.tile([P, 2], f32)
            nc.sync.dma_start(out=b_col[:], in_=b_d.rearrange("(h p) -> p h", p=P))
            g_col = consts.tile([P, 2], f32)
            nc.sync.dma_start(out=g_col[:], in_=g_d.rearrange("(h p) -> p h", p=P))
            be_col = consts.tile([P, 2], f32)
            nc.sync.dma_start(out=be_col[:], in_=be_d.rearrange("(h p) -> p h", p=P))

            # W^T blocks [fin128, fout128] (c = fin chunk, a = fout half), f32r
            wT = consts.tile([P, 2, 2, P], f32r)
            for a in range(2):
                for c in range(2):
                    tp = ps_o.tile([P, P], f32, tag="o", name="wtp")
                    nc.tensor.transpose(tp[:], w_sb[:, a, c * P:(c + 1) * P], ident_f[:])
                    nc.vector.tensor_copy(wT[:, c, a, :], tp[:])

            # per-sample state holders
            ys = [[[None] * GROUPS for _ in range(2)] for _ in range(SPB)]
            s2d4 = [None] * SPB
            kbc = [None] * SPB
            stats_t = [[None] * 2 for _ in range(SPB)]
            for s in range(SPB):
                for a in range(2):
                    stats_t[s][a] = stats.tile(
                        [P, GROUPS, 6], f32, tag=f"st{s}{a}", name=f"st{s}{a}"
                    )

            xhold = [None]

            def emit_group(s, g):
                tok0 = s * N + g * 512
                if g % 2 == 0:
                    if s == 0 and g in xpre:
                        x_nat = xpre.pop(g)
                    else:
                        x_nat = xin.tile([P, 8, F], f32r, tag="xnat")
                        nc.gpsimd.dma_start(
                            out=x_nat[:],
                            in_=x_d[tok0:tok0 + 1024, :].rearrange(
                                "(t p) f -> p t f", p=P),
                        )
                    xhold[0] = x_nat
                else:
                    x_nat = xhold[0]
                toff = (g % 2) * 4

                xt_sb = xtp.tile([P, 2, 512], f32r, tag="xt")
                for c in range(2):
                    pxt = ps_xt.tile([P, 512], f32r, tag="xt")
                    for t in range(4):
                        nc.tensor.matmul(
                            pxt[:, t * P:(t + 1) * P],
                            x_nat[:, toff + t, c * P:(c + 1) * P],
                            ident_r[:],
                            is_transpose=True,
                            start=(t == 0),
                            stop=(t == 3),
                        )
                    nc.scalar.copy(out=xt_sb[:, c, :], in_=pxt[:])

                for a in range(2):
                    yps = ps_y.tile([P, 512], f32, tag="y")
                    for c in range(2):
                        nc.tensor.matmul(
                            yps[:], wT[:, c, a, :], xt_sb[:, c, :],
                            start=(c == 0), stop=(c == 1),
                        )
                    nc.vector.bn_stats(
                        out=stats_t[s][a][:, g, :], in_=yps[:],
                    )
                    yt = ypool.tile([P, 512], f32r, tag="y")
                    nc.scalar.copy(out=yt[:], in_=yps[:])
                    ys[s][a][g] = yt

            kcol_h = [None] * SPB
            shift_h = [None] * SPB

            def emit_finalize(s):
                k_cols = []
                s2_cols = []
                sh_cols = []
                for a in range(2):
                    mv = stats.tile([P, 2], f32, tag="fmv")
                    nc.vector.bn_aggr(out=mv[:], in_=stats_t[s][a][:])
                    std = stats.tile([P, 1], f32, tag="fstd")
                    nc.scalar.activation(
                        out=std[:], in_=mv[:, 1:2], func=AF.Sqrt,
                        bias=eps_t[:], scale=1.0,
                    )
                    k = stats.tile([P, 1], f32, tag="fk")
                    nc.vector.reciprocal(out=k[:], in_=std[:])
                    nc.vector.tensor_mul(out=k[:], in0=k[:], in1=g_col[:, a:a + 1])
                    if s == 0:
                        rk = stats.tile([P, 1], f32, tag="frk")
                        nc.vector.reciprocal(out=rk[:], in_=k[:])
                        s2 = stats.tile([P, 1], f32, tag="fs2")
                        nc.vector.tensor_mul(out=s2[:], in0=be_col[:, a:a + 1], in1=rk[:])
                        nc.vector.tensor_sub(out=s2[:], in0=s2[:], in1=mv[:, 0:1])
                        s2_cols.append(s2)
                    else:
                        # shift = beta - mean*k for the ACT-normalize path
                        sh = stats.tile([P, 1], f32, tag="fsh")
                        nc.vector.tensor_mul(out=sh[:], in0=mv[:, 0:1], in1=k[:])
                        nc.vector.tensor_sub(out=sh[:], in0=be_col[:, a:a + 1], in1=sh[:])
                        sh_cols.append(sh)
                    k_cols.append(k)
                kcol_h[s] = k_cols
                shift_h[s] = sh_cols
                if s != 0:
                    return

                kd = misc.tile([P, 4, P], f32r, tag="kd4")
                s2d = misc.tile([P, 4, P], f32r, tag="s2d4")
                for q in range(4):
                    nc.vector.tensor_scalar_mul(
                        out=kd[:, q, :], in0=ident_r[:], scalar1=k_cols[q % 2][:],
                    )
                    nc.vector.tensor_scalar_mul(
                        out=s2d[:, q, :], in0=ident_r[:], scalar1=s2_cols[q % 2][:],
                    )
                kb_ps = ps_o.tile([P, 512], f32, tag="o", name="kbps")
                nc.tensor.matmul(
                    kb_ps[:], ones_r[:], kd.rearrange("p a f -> p (a f)"),
                    start=True, stop=True,
                )
                kb = misc.tile([P, 512], f32, tag="kb")
                nc.vector.tensor_copy(kb[:], kb_ps[:])
                s2d4[s] = s2d
                kbc[s] = kb

            ohold = [None]
            ops_live = {}

            ysn = [[None] * GROUPS for _ in range(2)]

            def emit_out_T(s, j, src_tiles, close_group=False):
                # transposes for out tokens [s*N + j*256, +256); quarters (t, a)
                # sample-1 tail uses the then-idle ps_y banks: 3-deep rotation
                pool_ = ps_y if close_group else ps_o
                tg = "y" if close_group else "o"
                ops = pool_.tile([P, 512], f32r, tag=tg, name="ops")
                for q in range(4):
                    tb = 2 * j + (q >> 1)
                    a = q & 1
                    g, t = tb >> 2, tb & 3
                    nc.tensor.matmul(
                        ops[:, q * P:(q + 1) * P],
                        src_tiles[a][g][:, t * P:(t + 1) * P],
                        ident_r[:],
                        is_transpose=True,
                        start=(q == 0), stop=(close_group and q == 3),
                    )
                ops_live[(s, j)] = ops

            def emit_out_fin(s, j):
                ops = ops_live.pop((s, j))
                nc.tensor.matmul(
                    ops.bitcast(f32)[:], ones_r[:],
                    s2d4[s].rearrange("p a f -> p (a f)"),
                    start=False, stop=True,
                )
                if j % 2 == 0:
                    ohold[0] = outp.tile([P, 2, 512], f32, tag="o", name="osb")
                osb = ohold[0]
                half = j % 2
                nc.vector.tensor_mul(
                    out=osb[:, half, :], in0=ops.bitcast(f32)[:], in1=kbc[s][:]
                )
                if half == 1:
                    row0 = s * N + (j - 1) * 256
                    nc.sync.dma_start(
                        out=out_d[row0:row0 + 512, :].rearrange("(t p) f -> p t f", p=P),
                        in_=osb.rearrange("p h (t f) -> p (h t) f", f=F),
                    )

            def emit_out_fin_s1(s, j):
                ops = ops_live.pop((s, j))
                if j % 2 == 0:
                    ohold[0] = outp.tile([P, 2, 512], f32, tag="o", name="osb")
                osb = ohold[0]
                half = j % 2
                nc.vector.tensor_copy(out=osb[:, half, :], in_=ops.bitcast(f32)[:])
                if half == 1:
                    row0 = s * N + (j - 1) * 256
                    nc.sync.dma_start(
                        out=out_d[row0:row0 + 512, :].rearrange("(t p) f -> p t f", p=P),
                        in_=osb.rearrange("p h (t f) -> p (h t) f", f=F),
                    )

            # Sample-0 groups, then sample-1 groups interleaved with sample-0
            # output (keeps ACT FIFO from blocking on y-pool slots). Out-phase
            # transposes run 2 chunks ahead of the bias-matmul so the PE stays
            # busy while the finalize chain computes k/s2.
            NJ = 2 * GROUPS
            for g in range(GROUPS):
                emit_group(0, g)
            emit_finalize(0)
            emit_out_T(0, 0, ys[0])
            emit_out_T(0, 1, ys[0])
            for g in range(GROUPS):
                emit_group(1, g)
                for jj in (2 * g, 2 * g + 1):
                    emit_out_fin(0, jj)
                    if jj + 2 < NJ:
                        emit_out_T(0, jj + 2, ys[0])
            emit_finalize(1)
            # sample 1: normalize y^T on the (now idle) ACT engine, then the
            # output chunks are pure transposes + copy evac (no bias matmul).
            for g in range(GROUPS):
                for a in range(2):
                    yn = ypool.tile([P, 512], f32r, tag="y", name=f"yn{g}{a}")
                    nc.scalar.activation(
                        out=yn[:], in_=ys[1][a][g][:], func=AF.Identity,
                        bias=shift_h[1][a][:], scale=kcol_h[1][a][:],
                    )
                    ysn[a][g] = yn
            emit_out_T(1, 0, ysn, close_group=True)
            emit_out_T(1, 1, ysn, close_group=True)
            emit_out_T(1, 2, ysn, close_group=True)
            for j in range(NJ):
                emit_out_fin_s1(1, j)
                if j + 3 < NJ:
                    emit_out_T(1, j + 3, ysn, close_group=True)

    nc.compile()
    return nc


def _get_nc():
    if "nc" not in _NC_CACHE:
        _NC_CACHE["nc"] = _build_nc()
    return _NC_CACHE["nc"]


def kernel(x, W, b, gamma, beta):
    from concourse.bass_utils import run_bass_kernel_spmd

    x = np.asarray(x, dtype=np.float32)
    W = np.asarray(W, dtype=np.float32)
    b = np.asarray(b, dtype=np.float32)
    gamma = np.asarray(gamma, dtype=np.float32)
    beta = np.asarray(beta, dtype=np.float32)

    nc = _get_nc()
    shards = x.reshape(CORES, TOK, F)
    in_maps = [
        {
            "x": np.ascontiguousarray(shards[i]),
            "w": W, "b": b, "gamma": gamma, "beta": beta,
        }
        for i in range(CORES)
    ]
    try:
        res = run_bass_kernel_spmd(nc, in_maps, core_ids=list(range(CORES)))
    except Exception:
        # One retry: a previous crashed run can leave a core wedged.
        res = run_bass_kernel_spmd(nc, in_maps, core_ids=list(range(CORES)))
    out = np.stack([res.results[i]["out"] for i in range(CORES)])
    return out.reshape(B, N, F).astype(np.float32)


if __name__ == "__main__":
    rng = np.random.default_rng(0)
    x = rng.standard_normal((B, N, F), dtype=np.float32)
    W = ((rng.random((F, F), dtype=np.float32) - 0.5) / 8).astype(np.float32)
    b = ((rng.random(F, dtype=np.float32) - 0.5) / 8).astype(np.float32)
    gamma = np.ones(F, np.float32)
    beta = np.zeros(F, np.float32)
    out = kernel(x=x, W=W, b=b, gamma=gamma, beta=beta)
    y = x @ W.T + b
    mean = y.mean(axis=1, keepdims=True)
    var = ((y - mean) ** 2).mean(axis=1, keepdims=True)
    ref = (y - mean) / np.sqrt(var + EPS) * gamma + beta
    err = np.abs(out - ref).max()
    print("maxabs err:", err, "rel:", err / np.abs(ref).max())

